# revision 45
# baseline (speedup 1.0000x reference)
"""Trainium2 Bass kernel for nn_PlasticityModelMoE (8-core SPMD).

Strategy (v3):
  Units tensor-parallel phase 1 (256 units/core): w_mod = w*sigmoid(delay)
  computed on device in bf16 with conn*mask and the fp8 scale folded in;
  branch+gate logits via fp8 DoubleRow matmuls (bias added through a bf16
  ones-row step), gate softmax, z, relu, degree-4 poly blend (x32 fp8
  scale baked into the coefficients). x arrives pre-transposed in fp8
  (host-side dtype/layout marshalling, like the baseline's b-major weight
  reshape) so the PE does no transposes and no casting DMAs are needed.
  blendT staged per 512-col chunk via the XBAR DMA transpose, AllGather
  in fp8 (1MB out), phase 3 (logitsT = read_W x blendT) in fp8 DoubleRow,
  exp with descale+bias, phase 4 (E @ [mem|1], bf16) interleaved into the
  phase-1 tile loop, f32 ReduceScatter per 256-row piece (8 total),
  per-piece epilogue."""
import ml_dtypes
import numpy as np
from contextlib import ExitStack

import concourse.bass as bass
import concourse.mybir as mybir
import concourse.tile as tile
from concourse import bacc
from concourse.bass_utils import run_bass_kernel_spmd
from concourse.masks import make_identity

F32 = mybir.dt.float32
BF16 = mybir.dt.bfloat16
FP8 = mybir.dt.float8e4
AF = mybir.ActivationFunctionType
ALU = mybir.AluOpType
AX = mybir.AxisListType
DR = mybir.MatmulPerfMode.DoubleRow

KC = 8
N, D, U, NB, M, MD = 2048, 1024, 2048, 4, 8192, 1024
US = U // KC          # 256 units per core
MS = M // KC          # 1024 memory rows per core
NS = N // KC          # 256 output rows per core
NT = N // 128         # 16 batch tiles
DK = D // 128         # 8 k-tiles over D
UK = U // 128         # 16 k-tiles over U
MK = MS // 128        # 8 k-tiles over memory shard
UBF = US * NB         # 1024 branch columns per core
COLS = UBF + NB       # 1028
S_W = 512.0           # fp8 scale for w_mod / gate_W
S_BL = 32.0           # fp8 scale for blend
S_RW = 256.0          # fp8 scale for read_W

_CMAT = np.array([
    [5.0000238e-01, 2.4987496e-01, 1.0582031e-03, -2.4046743e-02, 4.1678566e-03],
    [0.0, 1.0, 0.0, 0.0, 0.0],
    [-7.2632770e-06, 9.9976927e-01, 9.2018498e-03, -3.9401752e-01, 1.4669961e-01],
    [0.0, 1.0, 0.0, 0.0, 0.0],
    [8.6798245e-06, 4.9957812e-01, 2.5321743e-01, -8.1970906e-03, -1.3558048e-02],
    [3.9388153e-05, 4.9807969e-01, 4.1364601e-01, -3.7666172e-02, -3.2796454e-02],
    [0.0, 1.0507009873554805, 0.0, 0.0, 0.0],
    [3.1482985e-05, 5.9846270e-01, 3.3178753e-01, -4.6201140e-02, -1.9015398e-02],
    [0.0, 0.0, 0.0, 0.0, 0.0],
], dtype=np.float32)
# the poly is evaluated at a_scaled = S_W*a (the branch matmul's fp8 scale is
# never divided out); fold S_BL and the per-degree 1/S_W^k into the coefs
_CMAT = _CMAT * (S_BL / S_W ** np.arange(5, dtype=np.float32))[None, :]

# slot -> (chunk, mk_start, mk_end) phase-3 groups
P3_SCHED = {5: (0, 0, 3), 6: (0, 3, 6), 7: (0, 6, 8),
            9: (1, 0, 3), 10: (1, 3, 6), 11: (1, 6, 8),
            13: (2, 0, 3), 14: (2, 3, 6), 15: (2, 6, 8)}
# slot -> (chunk, sj) phase-4 subtiles
P4_SCHED = {8: (0, 0), 9: (0, 1), 10: (0, 2), 11: (0, 3),
            12: (1, 0), 13: (1, 1), 14: (1, 2), 15: (1, 3)}

_cache = {}


def _build():
    nc = bacc.Bacc(num_devices=KC)

    xt8_d = nc.dram_tensor("xt8", [D, N], FP8, kind="ExternalInput")
    w16_d = nc.dram_tensor("w16", [D, COLS], BF16, kind="ExternalInput")
    d16_d = nc.dram_tensor("d16", [D, UBF], BF16, kind="ExternalInput")
    bias_d = nc.dram_tensor("bias", [COLS], F32, kind="ExternalInput")
    na_d = nc.dram_tensor("na", [U], F32, kind="ExternalInput")
    cw1_d = nc.dram_tensor("cw1", [U, 32], F32, kind="ExternalInput")
    cb1_d = nc.dram_tensor("cb1", [32], F32, kind="ExternalInput")
    cw2_d = nc.dram_tensor("cw2", [32, US], F32, kind="ExternalInput")
    cb2_d = nc.dram_tensor("cb2", [US], F32, kind="ExternalInput")
    mask_d = nc.dram_tensor("maskv", [US], F32, kind="ExternalInput")
    actw_d = nc.dram_tensor("actw", [9], F32, kind="ExternalInput")
    rw8_d = nc.dram_tensor("rw8", [U, MS], FP8, kind="ExternalInput")
    rb_d = nc.dram_tensor("rb", [MS], F32, kind="ExternalInput")
    mem16_d = nc.dram_tensor("mem16", [MS, MD], BF16, kind="ExternalInput")
    cmat_d = nc.dram_tensor("cmat", [9, 5], F32, kind="ExternalInput")
    y_d = nc.dram_tensor("y", [NS, MD], F32, kind="ExternalOutput")

    with tile.TileContext(nc) as tc, ExitStack() as ctx:
        consts = ctx.enter_context(tc.tile_pool(name="consts", bufs=1))
        xpool = ctx.enter_context(tc.tile_pool(name="xpool", bufs=1))
        wpool = ctx.enter_context(tc.tile_pool(name="wpool", bufs=1))
        sgp = ctx.enter_context(tc.tile_pool(name="sgp", bufs=2))
        p34 = ctx.enter_context(tc.tile_pool(name="p34", bufs=1))
        p3p = ctx.enter_context(tc.tile_pool(name="p3p", bufs=2))
        blendp = ctx.enter_context(tc.tile_pool(name="blendp", bufs=2))
        p4p = ctx.enter_context(tc.tile_pool(name="p4p", bufs=2))
        dram_ag = ctx.enter_context(tc.tile_pool(name="dram_ag", bufs=1, space="DRAM"))
        dram_rs = ctx.enter_context(tc.tile_pool(name="dram_rs", bufs=1, space="DRAM"))
        # PSUM budget (8 banks): "br" [128,1024] f32 = 2 banks x 3 bufs = 6
        # (phase-1 br_ps and phase-4 r_ps alternate); "tr" <=512 = 1 x 2 = 2
        # (setup, gate logits, blendT transposes, phase-3 l_ps, s column).
        psum = ctx.enter_context(tc.tile_pool(name="psum", bufs=2, space="PSUM"))

        # ---------------- tiny consts ----------------
        ones_lhs = consts.tile([1, 128], BF16)
        nc.vector.memset(ones_lhs, 1.0)
        ones_f = consts.tile([1, 128], F32)
        nc.vector.memset(ones_f, 1.0)
        idf1 = consts.tile([1, 1], F32)
        nc.vector.memset(idf1, 1.0)
        idf = consts.tile([128, 128], F32)
        make_identity(nc, idf)
        idb = consts.tile([128, 128], BF16)
        nc.any.tensor_copy(idb, idf)

        # small loads first on sync queue
        na_sb = consts.tile([128, UK], F32)
        nc.sync.dma_start(out=na_sb, in_=na_d.ap().rearrange("(t p) -> p t", p=128))
        cw1_sb = consts.tile([128, UK, 32], F32)
        nc.sync.dma_start(out=cw1_sb,
                          in_=cw1_d.ap().rearrange("(t p) c -> p t c", p=128))
        cb1_sb = consts.tile([1, 32], F32)
        nc.sync.dma_start(out=cb1_sb, in_=cb1_d.ap()[None])
        cw2_sb = consts.tile([32, US], F32)
        nc.sync.dma_start(out=cw2_sb, in_=cw2_d[:, :])
        cb2_sb = consts.tile([1, US], F32)
        nc.sync.dma_start(out=cb2_sb, in_=cb2_d.ap()[None])
        mask_sb = consts.tile([1, US], F32)
        nc.sync.dma_start(out=mask_sb, in_=mask_d.ap()[None])
        aw = consts.tile([1, 9], F32)
        nc.sync.dma_start(out=aw, in_=actw_d.ap()[None])
        cmat_sb = consts.tile([9, 5], F32)
        nc.sync.dma_start(out=cmat_sb, in_=cmat_d[:, :])
        bias_f = consts.tile([1, COLS], F32)
        nc.sync.dma_start(out=bias_f, in_=bias_d.ap()[None])
        rb_sb = consts.tile([128, MK], F32)
        nc.sync.dma_start(out=rb_sb, in_=rb_d.ap().rearrange("(t p) -> p t", p=128))

        # ---------------- connectivity -> cm scale row ----------------
        h_ps = psum.tile([1, 512], F32, tag="tr")
        for t in range(UK):
            nc.tensor.matmul(h_ps[:, 0:32], na_sb[:, t:t + 1], cw1_sb[:, t, :],
                             start=(t == 0), stop=(t == UK - 1))
        h_pre = consts.tile([1, 32], F32)
        nc.vector.tensor_add(h_pre, h_ps[:, 0:32], cb1_sb)
        h_sb = consts.tile([1, 32], F32)
        nc.scalar.activation(h_sb, h_pre, AF.Relu)
        hT_ps = psum.tile([32, 1], F32, tag="tr")
        nc.tensor.transpose(hT_ps, h_sb, idf1)
        hT_sb = consts.tile([32, 1], F32)
        nc.any.tensor_copy(hT_sb, hT_ps)
        cn_ps = psum.tile([1, 512], F32, tag="tr")
        nc.tensor.matmul(cn_ps[:, 0:US], hT_sb, cw2_sb, start=True, stop=True)
        cn_pre = consts.tile([1, US], F32)
        nc.vector.tensor_add(cn_pre, cn_ps[:, 0:US], cb2_sb)
        cn_sig = consts.tile([1, US], F32)
        nc.scalar.activation(cn_sig, cn_pre, AF.Sigmoid)
        cm_row = consts.tile([1, US], F32)
        nc.vector.tensor_mul(cm_row, cn_sig, mask_sb)
        # cm4_row = S_W * cm tiled 4x (b-major columns)
        cm4_row = consts.tile([1, UBF], F32)
        for b in range(NB):
            nc.any.tensor_scalar_mul(cm4_row[:, b * US:(b + 1) * US], cm_row, S_W)
        # broadcast to 128 partitions via ones matmul
        cm_bc = consts.tile([128, UBF], F32)
        for h in range(2):
            cm_ps = psum.tile([128, 512], F32, tag="tr")
            nc.tensor.matmul(cm_ps, ones_f, cm4_row[:, h * 512:(h + 1) * 512],
                             start=True, stop=True)
            nc.any.tensor_copy(cm_bc[:, h * 512:(h + 1) * 512], cm_ps)
        # bias row scaled: branch cols *cm*S_W, gate cols *S_W -> bf16
        scale_row = consts.tile([1, COLS], F32)
        nc.any.tensor_copy(scale_row[:, 0:UBF], cm4_row)
        nc.vector.memset(scale_row[:, UBF:COLS], S_W)
        bias16 = consts.tile([1, COLS], BF16)
        nc.vector.tensor_mul(bias16, bias_f, scale_row)

        # ---------------- act_w softmax -> poly coefs (x S_BL) ------------
        aw_negmax = consts.tile([1, 1], F32)
        nc.vector.tensor_reduce(aw_negmax, aw, AX.X, ALU.max, negate=True)
        aw_exp = consts.tile([1, 9], F32)
        nc.scalar.activation(aw_exp, aw, AF.Exp, bias=aw_negmax)
        aw_sum = consts.tile([1, 1], F32)
        nc.vector.tensor_reduce(aw_sum, aw_exp, AX.X, ALU.add)
        aw_rec = consts.tile([1, 1], F32)
        nc.vector.reciprocal(aw_rec, aw_sum)
        wts_row = consts.tile([1, 9], F32)
        nc.vector.tensor_scalar_mul(wts_row, aw_exp, aw_rec)
        wtsT_ps = psum.tile([9, 1], F32, tag="tr")
        nc.tensor.transpose(wtsT_ps, wts_row, idf1)
        wtsT = consts.tile([9, 1], F32)
        nc.any.tensor_copy(wtsT, wtsT_ps)
        cw_ps = psum.tile([1, 512], F32, tag="tr")
        nc.tensor.matmul(cw_ps[:, 0:5], wtsT, cmat_sb, start=True, stop=True)
        cw_row = consts.tile([1, 5], F32)
        nc.any.tensor_copy(cw_row, cw_ps[:, 0:5])
        bc_ps = psum.tile([128, 512], F32, tag="tr")
        nc.tensor.matmul(bc_ps[:, 0:5], ones_f, cw_row, start=True, stop=True)
        coefs = consts.tile([128, 5], F32)
        nc.any.tensor_copy(coefs, bc_ps[:, 0:5])

        # ---------------- big loads (all pre-marshalled, no casts) --------
        xT8 = xpool.tile([128, DK, N], FP8)
        for dk in range(DK):
            nc.sync.dma_start(out=xT8[:, dk, :],
                              in_=xt8_d[dk * 128:(dk + 1) * 128, :])
        w16 = wpool.tile([128, DK, COLS], BF16)
        d16 = wpool.tile([128, DK, UBF], BF16)
        for dk in range(DK):
            nc.sync.dma_start(out=w16[:, dk, :],
                              in_=w16_d[dk * 128:(dk + 1) * 128, :])
            nc.scalar.dma_start(out=d16[:, dk, :],
                                in_=d16_d[dk * 128:(dk + 1) * 128, :])
        rw8 = p34.tile([128, UK, MS], FP8)
        for uk in range(UK):
            nc.scalar.dma_start(out=rw8[:, uk, :],
                                in_=rw8_d[uk * 128:(uk + 1) * 128, :])
        mem_sb = p34.tile([128, MK, MD + 1], BF16)
        for mk in range(MK):
            nc.scalar.dma_start(out=mem_sb[:, mk, 0:MD],
                                in_=mem16_d[mk * 128:(mk + 1) * 128, :])
        nc.vector.memset(mem_sb[:, :, MD:MD + 1], 1.0)

        # ---------------- w_mod prep (fp8, conn*mask*S_W folded) ----------
        wmod8 = wpool.tile([128, DK, COLS], FP8)
        for dk in range(DK):
            sig_b = sgp.tile([128, UBF], BF16, tag="sg", name=f"sg{dk}")
            nc.scalar.activation(sig_b, d16[:, dk, :], AF.Sigmoid)
            wcm = sgp.tile([128, UBF], BF16, tag="wc", name=f"wc{dk}")
            nc.any.tensor_mul(wcm, w16[:, dk, 0:UBF], cm_bc)
            nc.any.tensor_mul(wmod8[:, dk, 0:UBF], wcm, sig_b)
            nc.any.tensor_scalar_mul(wmod8[:, dk, UBF:COLS],
                                     w16[:, dk, UBF:COLS], S_W)

        # ---------------- pipelined main loop -----------------------------
        bTs, expTs, rs_outs = {}, {}, {}
        blendT_cur = [None]
        n_epi = [0]

        def phase1_tile(i):
            nsl = slice(i * 128, (i + 1) * 128)
            if i % 4 == 0:
                blendT_cur[0] = p3p.tile([128, 2, 512], FP8, tag="b8",
                                         name="bT8")
            br_ps = psum.tile([128, UBF], F32, tag="br", name="br_ps", bufs=3)
            for (c0, c1) in [(0, 512), (512, 1024)]:
                for j in range(DK // 2):
                    nc.tensor.matmul(br_ps[:, c0:c1],
                                     xT8[:, 2 * j:2 * j + 2, nsl],
                                     wmod8[:, 2 * j:2 * j + 2, c0:c1],
                                     start=(j == 0), stop=False, perf_mode=DR)
                nc.tensor.matmul(br_ps[:, c0:c1], ones_lhs, bias16[:, c0:c1],
                                 start=False, stop=True)
            gate_ps = psum.tile([128, NB], F32, tag="tr", name="gate_ps")
            for j in range(DK // 2):
                nc.tensor.matmul(gate_ps, xT8[:, 2 * j:2 * j + 2, nsl],
                                 wmod8[:, 2 * j:2 * j + 2, UBF:COLS],
                                 start=(j == 0), stop=False, perf_mode=DR)
            nc.tensor.matmul(gate_ps, ones_lhs, bias16[:, UBF:COLS],
                             start=False, stop=True)
            # free the PSUM bank fast: bf16 copy of branch cols (vector) +
            # gate exp straight out of PSUM (scalar)
            br_sb = blendp.tile([128, UBF], BF16, tag="brs")
            nc.vector.tensor_copy(br_sb, br_ps)
            # gate softmax (no max-sub; logits are small), 1/S_W folded in
            g_exp = blendp.tile([128, NB], F32, tag="g1")
            nc.scalar.activation(g_exp, gate_ps, AF.Exp, scale=1.0 / S_W)
            g_sum = blendp.tile([128, 1], F32, tag="g2")
            nc.vector.tensor_reduce(g_sum, g_exp, AX.X, ALU.add)
            g_rec = blendp.tile([128, 1], F32, tag="g4")
            nc.vector.reciprocal(g_rec, g_sum)
            gate_sb = blendp.tile([128, NB], F32, tag="g5")
            nc.vector.tensor_scalar_mul(gate_sb, g_exp, g_rec)
            # z = sum_b gate_b * branch_b (S_W cancels via g_rec)
            zt0 = blendp.tile([128, US], BF16, tag="t0")
            nc.any.tensor_scalar_mul(zt0, br_sb[:, 0:US], gate_sb[:, 0:1])
            zt1 = blendp.tile([128, US], BF16, tag="t1")
            nc.any.tensor_scalar_mul(zt1, br_sb[:, US:2 * US],
                                     gate_sb[:, 1:2])
            zt2 = blendp.tile([128, US], BF16, tag="t2")
            nc.any.tensor_scalar_mul(zt2, br_sb[:, 2 * US:3 * US],
                                     gate_sb[:, 2:3])
            zt3 = blendp.tile([128, US], BF16, tag="t3")
            nc.any.tensor_scalar_mul(zt3, br_sb[:, 3 * US:4 * US],
                                     gate_sb[:, 3:4])
            z01 = blendp.tile([128, US], BF16, tag="t0")
            nc.any.tensor_add(z01, zt0, zt1)
            z23 = blendp.tile([128, US], BF16, tag="t2")
            nc.any.tensor_add(z23, zt2, zt3)
            z_sb = blendp.tile([128, US], BF16, tag="t1")
            nc.any.tensor_add(z_sb, z01, z23)
            a_sb = blendp.tile([128, US], BF16, tag="ta")
            nc.any.tensor_scalar_max(a_sb, z_sb, 0.0)
            # blend via degree-4 Horner (coefs pre-scaled by S_BL)
            hp = blendp.tile([128, US], BF16, tag="t2")
            nc.any.tensor_scalar(hp, a_sb, coefs[:, 4:5], coefs[:, 3:4],
                                 ALU.mult, ALU.add)
            hq = blendp.tile([128, US], BF16, tag="t3")
            nc.any.tensor_mul(hq, hp, a_sb)
            hr = blendp.tile([128, US], BF16, tag="t2")
            nc.any.tensor_scalar_add(hr, hq, coefs[:, 2:3])
            hs = blendp.tile([128, US], BF16, tag="t3")
            nc.any.tensor_mul(hs, hr, a_sb)
            ht = blendp.tile([128, US], BF16, tag="t2")
            nc.any.tensor_scalar_add(ht, hs, coefs[:, 1:2])
            hu = blendp.tile([128, US], BF16, tag="t3")
            nc.any.tensor_mul(hu, ht, a_sb)
            blend16 = blendp.tile([128, US], BF16, tag="bb")
            nc.any.tensor_scalar_add(blend16, hu, coefs[:, 0:1])
            csl = slice((i % 4) * 128, (i % 4 + 1) * 128)
            with tc.high_priority():
                for uh in range(2):
                    trb_ps = psum.tile([128, 128], BF16, tag="tr", name="trb")
                    nc.tensor.transpose(trb_ps,
                                        blend16[:, uh * 128:(uh + 1) * 128],
                                        idb)
                    nc.scalar.activation(blendT_cur[0][:, uh, csl], trb_ps,
                                         AF.Copy)

        def issue_ag(ch):
            with tc.high_priority():
                agi = dram_ag.tile([US, 512], FP8, name=f"agi{ch}",
                                   tag=f"agi{ch}")
                nc.scalar.dma_start(
                    out=agi.rearrange("(uh p) c -> p uh c", p=128),
                    in_=blendT_cur[0])
                ago = dram_ag.tile([U, 512], FP8, name=f"ago{ch}",
                                   tag=f"ago{ch}", addr_space="Shared")
                nc.gpsimd.collective_compute(
                    "AllGather", ALU.bypass, replica_groups=[list(range(KC))],
                    ins=[agi.opt()], outs=[ago.opt()])
                bT = p3p.tile([128, UK, 512], FP8, tag="bT", name="bT")
                for uk in range(UK):
                    nc.sync.dma_start(out=bT[:, uk, :],
                                      in_=ago[uk * 128:(uk + 1) * 128, :])
            bTs[ch] = bT
            expTs[ch] = p3p.tile([128, MK, 512], BF16, tag="expT", name="expT")

        def phase3_groups(ch, m0, m1):
            bT, expT = bTs[ch], expTs[ch]
            for mk in range(m0, m1):
                l_ps = psum.tile([128, 512], F32, tag="tr", name="l_ps")
                for j in range(UK // 2):
                    nc.tensor.matmul(l_ps,
                                     rw8[:, 2 * j:2 * j + 2,
                                         mk * 128:(mk + 1) * 128],
                                     bT[:, 2 * j:2 * j + 2, :],
                                     start=(j == 0), stop=(j == UK // 2 - 1),
                                     perf_mode=DR)
                nc.scalar.activation(expT[:, mk, :], l_ps, AF.Exp,
                                     bias=rb_sb[:, mk:mk + 1],
                                     scale=1.0 / (S_BL * S_RW))

        def emit_epilogue(p):
            e_f = p4p.tile([64, MD + 1], F32, tag="ef", name="e_f")
            nc.gpsimd.dma_start(out=e_f, in_=rs_outs[p][:, :])
            s_rec = p4p.tile([64, 1], F32, tag="sr", name="s_rec", bufs=1)
            nc.vector.reciprocal(s_rec, e_f[:, MD:MD + 1])
            y_t = p4p.tile([64, MD], F32, tag="yt", name="y_t", bufs=1)
            nc.any.tensor_scalar_mul(y_t, e_f[:, 0:MD], s_rec)
            nc.gpsimd.dma_start(out=y_d[p * 64:(p + 1) * 64, :], in_=y_t)

        def phase4_sj(ch, sj):
            jsl = slice(sj * 128, (sj + 1) * 128)
            expT = expTs[ch]
            r_ps = psum.tile([128, UBF], F32, tag="br", name="r_ps", bufs=3)
            for (c0, c1) in [(0, 512), (512, 1024)]:
                for mk in range(MK):
                    nc.tensor.matmul(r_ps[:, c0:c1], expT[:, mk, jsl],
                                     mem_sb[:, mk, c0:c1],
                                     start=(mk == 0), stop=(mk == MK - 1))
            s_ps = psum.tile([128, 1], F32, tag="tr", name="s_ps")
            for mk in range(MK):
                nc.tensor.matmul(s_ps, expT[:, mk, jsl],
                                 mem_sb[:, mk, MD:MD + 1],
                                 start=(mk == 0), stop=(mk == MK - 1))
            r_sb = p4p.tile([128, MD + 1], F32, tag="rsb", name="r_sb")
            nc.any.tensor_copy(r_sb[:, 0:MD], r_ps)
            nc.any.tensor_copy(r_sb[:, MD:MD + 1], s_ps)
            p = ch
            if sj == 0:
                rs_inj = dram_rs.tile([512, MD + 1], F32, name=f"rs_in{p}",
                                      tag=f"rsi{p}")
                phase4_sj.rs_inj = rs_inj
            rs_inj = phase4_sj.rs_inj
            nc.gpsimd.dma_start(out=rs_inj[sj * 128:(sj + 1) * 128, :],
                                in_=r_sb)
            if sj == 3:
                rs_out = dram_rs.tile([64, MD + 1], F32, name=f"rs_out{p}",
                                      tag=f"rso{p}")
                nc.gpsimd.collective_compute(
                    "ReduceScatter", ALU.add, replica_groups=[list(range(KC))],
                    ins=[rs_inj.opt()], outs=[rs_out.opt()])
                rs_outs[p] = rs_out
                if p >= 1:
                    emit_epilogue(n_epi[0])
                    n_epi[0] += 1

        for i in range(NT):
            phase1_tile(i)
            if i % 4 == 3:
                issue_ag(i // 4)
            if i in P3_SCHED:
                phase3_groups(*P3_SCHED[i])
            if i in P4_SCHED:
                phase4_sj(*P4_SCHED[i])

        # tail: phase4(ch2) fills the AG(3) flight window, then phase3/4(ch3)
        for sj in range(4):
            phase4_sj(2, sj)
        phase3_groups(3, 0, 8)
        for sj in range(4):
            phase4_sj(3, sj)
        while n_epi[0] < 4:
            emit_epilogue(n_epi[0])
            n_epi[0] += 1

    nc.compile()
    return nc


def _make_in_maps(inputs):
    FP8NP = ml_dtypes.float8_e4m3
    BF16NP = ml_dtypes.bfloat16
    x = np.asarray(inputs["x"], np.float32)
    w = np.asarray(inputs["w"], np.float32)
    delay = np.asarray(inputs["delay"], np.float32)
    b = np.asarray(inputs["b"], np.float32)
    gate_W = np.asarray(inputs["gate_W"], np.float32)
    gate_b = np.asarray(inputs["gate_b"], np.float32)
    na = np.ascontiguousarray(np.asarray(inputs["neuron_avg"], np.float32))
    cw1 = np.ascontiguousarray(np.asarray(inputs["conn_W1"], np.float32))
    cb1 = np.ascontiguousarray(np.asarray(inputs["conn_b1"], np.float32))
    cw2 = np.asarray(inputs["conn_W2"], np.float32)
    cb2 = np.asarray(inputs["conn_b2"], np.float32)
    mask = np.asarray(inputs["mask"], np.float32)
    actw = np.ascontiguousarray(np.asarray(inputs["act_w"], np.float32))
    read_W = np.asarray(inputs["read_W"], np.float32)
    read_b = np.asarray(inputs["read_b"], np.float32)
    mem = np.asarray(inputs["memory"], np.float32)

    xt8 = np.ascontiguousarray(
        np.clip(x.T, -240, 240)).astype(FP8NP)
    in_maps = []
    for k in range(KC):
        us, ue = k * US, (k + 1) * US
        ms, me = k * MS, (k + 1) * MS
        bias_row = np.concatenate([b[us:ue].T.reshape(-1),
                                   gate_b]).astype(np.float32)
        w16 = np.ascontiguousarray(np.concatenate(
            [w[:, us:ue, :].transpose(0, 2, 1).reshape(D, UBF), gate_W],
            axis=1)).astype(BF16NP)
        d16 = np.ascontiguousarray(
            delay[:, us:ue, :].transpose(0, 2, 1).reshape(D, UBF)
        ).astype(BF16NP)
        rw8 = np.ascontiguousarray(
            np.clip(read_W[:, ms:me] * S_RW, -240, 240)).astype(FP8NP)
        in_maps.append({
            "xt8": xt8,
            "w16": w16,
            "d16": d16,
            "bias": np.ascontiguousarray(bias_row),
            "na": na,
            "cw1": cw1,
            "cb1": cb1,
            "cw2": np.ascontiguousarray(cw2[:, us:ue]),
            "cb2": np.ascontiguousarray(cb2[us:ue]),
            "maskv": np.ascontiguousarray(mask[us:ue]),
            "actw": actw,
            "rw8": rw8,
            "rb": np.ascontiguousarray(read_b[ms:me]),
            "mem16": np.ascontiguousarray(mem[ms:me, :]).astype(BF16NP),
            "cmat": _CMAT,
        })
    return in_maps


def kernel(**inputs) -> np.ndarray:
    if "nc" not in _cache:
        _cache["nc"] = _build()
    nc = _cache["nc"]
    in_maps = _make_in_maps(inputs)
    res = run_bass_kernel_spmd(nc, in_maps, core_ids=list(range(KC)))
    out = np.empty((N, MD), np.float32)
    for k in range(KC):
        yk = res.results[k]["y"]
        for p in range(4):
            out[p * 512 + k * 64:p * 512 + (k + 1) * 64] = \
                yk[p * 64:(p + 1) * 64]
    return out


# revision 48
# speedup vs baseline: 1.0398x; 1.0398x over previous
"""Trainium2 Bass kernel for nn_PlasticityModelMoE (8-core SPMD).

Strategy (v3):
  Units tensor-parallel phase 1 (256 units/core): w_mod = w*sigmoid(delay)
  computed on device in bf16 with conn*mask and the fp8 scale folded in;
  branch+gate logits via fp8 DoubleRow matmuls (bias added through a bf16
  ones-row step), gate softmax, z, relu, degree-4 poly blend (x32 fp8
  scale baked into the coefficients). x arrives pre-transposed in fp8
  (host-side dtype/layout marshalling, like the baseline's b-major weight
  reshape) so the PE does no transposes and no casting DMAs are needed.
  blendT staged per 512-col chunk via the XBAR DMA transpose, AllGather
  in fp8 (1MB out), phase 3 (logitsT = read_W x blendT) in fp8 DoubleRow,
  exp with descale+bias, phase 4 (E @ [mem|1], bf16) interleaved into the
  phase-1 tile loop, f32 ReduceScatter per 256-row piece (8 total),
  per-piece epilogue."""
import ml_dtypes
import numpy as np
from contextlib import ExitStack

import concourse.bass as bass
import concourse.mybir as mybir
import concourse.tile as tile
from concourse import bacc
from concourse.bass_utils import run_bass_kernel_spmd
from concourse.masks import make_identity

F32 = mybir.dt.float32
BF16 = mybir.dt.bfloat16
FP8 = mybir.dt.float8e4
AF = mybir.ActivationFunctionType
ALU = mybir.AluOpType
AX = mybir.AxisListType
DR = mybir.MatmulPerfMode.DoubleRow

KC = 8
N, D, U, NB, M, MD = 2048, 1024, 2048, 4, 8192, 1024
US = U // KC          # 256 units per core
MS = M // KC          # 1024 memory rows per core
NS = N // KC          # 256 output rows per core
NT = N // 128         # 16 batch tiles
DK = D // 128         # 8 k-tiles over D
UK = U // 128         # 16 k-tiles over U
MK = MS // 128        # 8 k-tiles over memory shard
UBF = US * NB         # 1024 branch columns per core
COLS = UBF + NB       # 1028
S_W = 512.0           # fp8 scale for w_mod / gate_W
S_BL = 32.0           # fp8 scale for blend
S_RW = 256.0          # fp8 scale for read_W

_CMAT = np.array([
    [5.0000238e-01, 2.4987496e-01, 1.0582031e-03, -2.4046743e-02, 4.1678566e-03],
    [0.0, 1.0, 0.0, 0.0, 0.0],
    [-7.2632770e-06, 9.9976927e-01, 9.2018498e-03, -3.9401752e-01, 1.4669961e-01],
    [0.0, 1.0, 0.0, 0.0, 0.0],
    [8.6798245e-06, 4.9957812e-01, 2.5321743e-01, -8.1970906e-03, -1.3558048e-02],
    [3.9388153e-05, 4.9807969e-01, 4.1364601e-01, -3.7666172e-02, -3.2796454e-02],
    [0.0, 1.0507009873554805, 0.0, 0.0, 0.0],
    [3.1482985e-05, 5.9846270e-01, 3.3178753e-01, -4.6201140e-02, -1.9015398e-02],
    [0.0, 0.0, 0.0, 0.0, 0.0],
], dtype=np.float32)
# the poly is evaluated at a_scaled = S_W*a (the branch matmul's fp8 scale is
# never divided out); fold S_BL and the per-degree 1/S_W^k into the coefs
_CMAT = _CMAT * (S_BL / S_W ** np.arange(5, dtype=np.float32))[None, :]

# slot -> (chunk, mk_start, mk_end) phase-3 groups
P3_SCHED = {5: (0, 0, 3), 6: (0, 3, 6), 7: (0, 6, 8),
            9: (1, 0, 3), 10: (1, 3, 6), 11: (1, 6, 8),
            13: (2, 0, 3), 14: (2, 3, 6), 15: (2, 6, 8)}
# slot -> (chunk, sj) phase-4 subtiles
P4_SCHED = {8: (0, 0), 9: (0, 1), 10: (0, 2), 11: (0, 3),
            12: (1, 0), 13: (1, 1), 14: (1, 2), 15: (1, 3)}

_cache = {}


def _build():
    nc = bacc.Bacc(num_devices=KC)

    xt8_d = nc.dram_tensor("xt8", [D, N], FP8, kind="ExternalInput")
    w16_d = nc.dram_tensor("w16", [D, COLS], BF16, kind="ExternalInput")
    d16_d = nc.dram_tensor("d16", [D, UBF], BF16, kind="ExternalInput")
    bias_d = nc.dram_tensor("bias", [COLS], F32, kind="ExternalInput")
    na_d = nc.dram_tensor("na", [U], F32, kind="ExternalInput")
    cw1_d = nc.dram_tensor("cw1", [U, 32], F32, kind="ExternalInput")
    cb1_d = nc.dram_tensor("cb1", [32], F32, kind="ExternalInput")
    cw2_d = nc.dram_tensor("cw2", [32, US], F32, kind="ExternalInput")
    cb2_d = nc.dram_tensor("cb2", [US], F32, kind="ExternalInput")
    mask_d = nc.dram_tensor("maskv", [US], F32, kind="ExternalInput")
    actw_d = nc.dram_tensor("actw", [9], F32, kind="ExternalInput")
    rw8_d = nc.dram_tensor("rw8", [U, MS], FP8, kind="ExternalInput")
    rb_d = nc.dram_tensor("rb", [MS], F32, kind="ExternalInput")
    mem16_d = nc.dram_tensor("mem16", [MS, MD], BF16, kind="ExternalInput")
    cmat_d = nc.dram_tensor("cmat", [9, 5], F32, kind="ExternalInput")
    y_d = nc.dram_tensor("y", [NS, MD], F32, kind="ExternalOutput")

    with tile.TileContext(nc) as tc, ExitStack() as ctx:
        consts = ctx.enter_context(tc.tile_pool(name="consts", bufs=1))
        xpool = ctx.enter_context(tc.tile_pool(name="xpool", bufs=1))
        wpool = ctx.enter_context(tc.tile_pool(name="wpool", bufs=1))
        sgp = ctx.enter_context(tc.tile_pool(name="sgp", bufs=2))
        p34 = ctx.enter_context(tc.tile_pool(name="p34", bufs=1))
        p3p = ctx.enter_context(tc.tile_pool(name="p3p", bufs=2))
        blendp = ctx.enter_context(tc.tile_pool(name="blendp", bufs=2))
        p4p = ctx.enter_context(tc.tile_pool(name="p4p", bufs=2))
        dram_ag = ctx.enter_context(tc.tile_pool(name="dram_ag", bufs=1, space="DRAM"))
        dram_rs = ctx.enter_context(tc.tile_pool(name="dram_rs", bufs=1, space="DRAM"))
        # PSUM budget (8 banks): "br" [128,1024] f32 = 2 banks x 3 bufs = 6
        # (phase-1 br_ps and phase-4 r_ps alternate); "tr" <=512 = 1 x 2 = 2
        # (setup, gate logits, blendT transposes, phase-3 l_ps, s column).
        psum = ctx.enter_context(tc.tile_pool(name="psum", bufs=2, space="PSUM"))

        # ---------------- tiny consts ----------------
        ones_lhs = consts.tile([1, 128], BF16)
        nc.vector.memset(ones_lhs, 1.0)
        ones_f = consts.tile([1, 128], F32)
        nc.vector.memset(ones_f, 1.0)
        idf1 = consts.tile([1, 1], F32)
        nc.vector.memset(idf1, 1.0)
        idf = consts.tile([128, 128], F32)
        make_identity(nc, idf)
        idb = consts.tile([128, 128], BF16)
        nc.any.tensor_copy(idb, idf)

        # small loads first on sync queue
        na_sb = consts.tile([128, UK], F32)
        nc.sync.dma_start(out=na_sb, in_=na_d.ap().rearrange("(t p) -> p t", p=128))
        cw1_sb = consts.tile([128, UK, 32], F32)
        nc.sync.dma_start(out=cw1_sb,
                          in_=cw1_d.ap().rearrange("(t p) c -> p t c", p=128))
        cb1_sb = consts.tile([1, 32], F32)
        nc.sync.dma_start(out=cb1_sb, in_=cb1_d.ap()[None])
        cw2_sb = consts.tile([32, US], F32)
        nc.sync.dma_start(out=cw2_sb, in_=cw2_d[:, :])
        cb2_sb = consts.tile([1, US], F32)
        nc.sync.dma_start(out=cb2_sb, in_=cb2_d.ap()[None])
        mask_sb = consts.tile([1, US], F32)
        nc.sync.dma_start(out=mask_sb, in_=mask_d.ap()[None])
        aw = consts.tile([1, 9], F32)
        nc.sync.dma_start(out=aw, in_=actw_d.ap()[None])
        cmat_sb = consts.tile([9, 5], F32)
        nc.sync.dma_start(out=cmat_sb, in_=cmat_d[:, :])
        bias_f = consts.tile([1, COLS], F32)
        nc.sync.dma_start(out=bias_f, in_=bias_d.ap()[None])
        rb_sb = consts.tile([128, MK], F32)
        nc.sync.dma_start(out=rb_sb, in_=rb_d.ap().rearrange("(t p) -> p t", p=128))

        # ---------------- connectivity -> cm scale row ----------------
        h_ps = psum.tile([1, 512], F32, tag="tr")
        for t in range(UK):
            nc.tensor.matmul(h_ps[:, 0:32], na_sb[:, t:t + 1], cw1_sb[:, t, :],
                             start=(t == 0), stop=(t == UK - 1))
        h_pre = consts.tile([1, 32], F32)
        nc.vector.tensor_add(h_pre, h_ps[:, 0:32], cb1_sb)
        h_sb = consts.tile([1, 32], F32)
        nc.scalar.activation(h_sb, h_pre, AF.Relu)
        hT_ps = psum.tile([32, 1], F32, tag="tr")
        nc.tensor.transpose(hT_ps, h_sb, idf1)
        hT_sb = consts.tile([32, 1], F32)
        nc.any.tensor_copy(hT_sb, hT_ps)
        cn_ps = psum.tile([1, 512], F32, tag="tr")
        nc.tensor.matmul(cn_ps[:, 0:US], hT_sb, cw2_sb, start=True, stop=True)
        cn_pre = consts.tile([1, US], F32)
        nc.vector.tensor_add(cn_pre, cn_ps[:, 0:US], cb2_sb)
        cn_sig = consts.tile([1, US], F32)
        nc.scalar.activation(cn_sig, cn_pre, AF.Sigmoid)
        cm_row = consts.tile([1, US], F32)
        nc.vector.tensor_mul(cm_row, cn_sig, mask_sb)
        # cm4_row = S_W * cm tiled 4x (b-major columns)
        cm4_row = consts.tile([1, UBF], F32)
        for b in range(NB):
            nc.any.tensor_scalar_mul(cm4_row[:, b * US:(b + 1) * US], cm_row, S_W)
        # broadcast to 128 partitions via ones matmul
        cm_bc = consts.tile([128, UBF], F32)
        for h in range(2):
            cm_ps = psum.tile([128, 512], F32, tag="tr")
            nc.tensor.matmul(cm_ps, ones_f, cm4_row[:, h * 512:(h + 1) * 512],
                             start=True, stop=True)
            nc.any.tensor_copy(cm_bc[:, h * 512:(h + 1) * 512], cm_ps)
        # bias row scaled: branch cols *cm*S_W, gate cols *S_W -> bf16
        scale_row = consts.tile([1, COLS], F32)
        nc.any.tensor_copy(scale_row[:, 0:UBF], cm4_row)
        nc.vector.memset(scale_row[:, UBF:COLS], S_W)
        bias16 = consts.tile([1, COLS], BF16)
        nc.vector.tensor_mul(bias16, bias_f, scale_row)

        # ---------------- act_w softmax -> poly coefs (x S_BL) ------------
        aw_negmax = consts.tile([1, 1], F32)
        nc.vector.tensor_reduce(aw_negmax, aw, AX.X, ALU.max, negate=True)
        aw_exp = consts.tile([1, 9], F32)
        nc.scalar.activation(aw_exp, aw, AF.Exp, bias=aw_negmax)
        aw_sum = consts.tile([1, 1], F32)
        nc.vector.tensor_reduce(aw_sum, aw_exp, AX.X, ALU.add)
        aw_rec = consts.tile([1, 1], F32)
        nc.vector.reciprocal(aw_rec, aw_sum)
        wts_row = consts.tile([1, 9], F32)
        nc.vector.tensor_scalar_mul(wts_row, aw_exp, aw_rec)
        wtsT_ps = psum.tile([9, 1], F32, tag="tr")
        nc.tensor.transpose(wtsT_ps, wts_row, idf1)
        wtsT = consts.tile([9, 1], F32)
        nc.any.tensor_copy(wtsT, wtsT_ps)
        cw_ps = psum.tile([1, 512], F32, tag="tr")
        nc.tensor.matmul(cw_ps[:, 0:5], wtsT, cmat_sb, start=True, stop=True)
        cw_row = consts.tile([1, 5], F32)
        nc.any.tensor_copy(cw_row, cw_ps[:, 0:5])
        bc_ps = psum.tile([128, 512], F32, tag="tr")
        nc.tensor.matmul(bc_ps[:, 0:5], ones_f, cw_row, start=True, stop=True)
        coefs = consts.tile([128, 5], F32)
        nc.any.tensor_copy(coefs, bc_ps[:, 0:5])

        # ---------------- big loads (all pre-marshalled, no casts) --------
        xT8 = xpool.tile([128, DK, N], FP8)
        for dk in range(DK):
            nc.sync.dma_start(out=xT8[:, dk, :],
                              in_=xt8_d[dk * 128:(dk + 1) * 128, :])
        w16 = wpool.tile([128, DK, COLS], BF16)
        d16 = wpool.tile([128, DK, UBF], BF16)
        for dk in range(DK):
            nc.sync.dma_start(out=w16[:, dk, :],
                              in_=w16_d[dk * 128:(dk + 1) * 128, :])
            nc.scalar.dma_start(out=d16[:, dk, :],
                                in_=d16_d[dk * 128:(dk + 1) * 128, :])
        rw8 = p34.tile([128, UK, MS], FP8)
        for uk in range(UK):
            nc.scalar.dma_start(out=rw8[:, uk, :],
                                in_=rw8_d[uk * 128:(uk + 1) * 128, :])
        mem_sb = p34.tile([128, MK, MD + 1], BF16)
        for mk in range(MK):
            nc.scalar.dma_start(out=mem_sb[:, mk, 0:MD],
                                in_=mem16_d[mk * 128:(mk + 1) * 128, :])
        nc.vector.memset(mem_sb[:, :, MD:MD + 1], 1.0)

        # ---------------- w_mod prep (fp8, conn*mask*S_W folded) ----------
        wmod8 = wpool.tile([128, DK, COLS], FP8)
        for dk in range(DK):
            sig_b = sgp.tile([128, UBF], BF16, tag="sg", name=f"sg{dk}")
            nc.scalar.activation(sig_b, d16[:, dk, :], AF.Sigmoid)
            wcm = sgp.tile([128, UBF], BF16, tag="wc", name=f"wc{dk}")
            nc.any.tensor_mul(wcm, w16[:, dk, 0:UBF], cm_bc)
            nc.any.tensor_mul(wmod8[:, dk, 0:UBF], wcm, sig_b)
            nc.any.tensor_scalar_mul(wmod8[:, dk, UBF:COLS],
                                     w16[:, dk, UBF:COLS], S_W)

        # ---------------- pipelined main loop -----------------------------
        bTs, expTs, rs_outs = {}, {}, {}
        blendT_cur = [None]
        n_epi = [0]

        def phase1_tile(i):
            nsl = slice(i * 128, (i + 1) * 128)
            if i % 4 == 0:
                blendT_cur[0] = p3p.tile([128, 2, 512], FP8, tag="b8",
                                         name="bT8")
            br_ps = psum.tile([128, COLS], F32, tag="br", name="br_ps")
            for (c0, c1) in [(0, 512), (512, 1024), (1024, 1028)]:
                for j in range(DK // 2):
                    nc.tensor.matmul(br_ps[:, c0:c1],
                                     xT8[:, 2 * j:2 * j + 2, nsl],
                                     wmod8[:, 2 * j:2 * j + 2, c0:c1],
                                     start=(j == 0), stop=False, perf_mode=DR)
                nc.tensor.matmul(br_ps[:, c0:c1], ones_lhs, bias16[:, c0:c1],
                                 start=False, stop=True)
            # free the PSUM bank fast: bf16 copy of branch cols (vector) +
            # gate exp straight out of PSUM (scalar)
            br_sb = blendp.tile([128, UBF], BF16, tag="brs")
            nc.vector.tensor_copy(br_sb, br_ps[:, 0:UBF])
            # gate softmax (no max-sub; logits are small), 1/S_W folded in
            g_exp = blendp.tile([128, NB], F32, tag="g1")
            nc.scalar.activation(g_exp, br_ps[:, UBF:COLS], AF.Exp,
                                 scale=1.0 / S_W)
            g_sum = blendp.tile([128, 1], F32, tag="g2")
            nc.vector.tensor_reduce(g_sum, g_exp, AX.X, ALU.add)
            g_rec = blendp.tile([128, 1], F32, tag="g4")
            nc.vector.reciprocal(g_rec, g_sum)
            gate_sb = blendp.tile([128, NB], F32, tag="g5")
            nc.vector.tensor_scalar_mul(gate_sb, g_exp, g_rec)
            # z = sum_b gate_b * branch_b (S_W cancels via g_rec)
            zt0 = blendp.tile([128, US], BF16, tag="t0")
            nc.any.tensor_scalar_mul(zt0, br_sb[:, 0:US], gate_sb[:, 0:1])
            zt1 = blendp.tile([128, US], BF16, tag="t1")
            nc.any.tensor_scalar_mul(zt1, br_sb[:, US:2 * US],
                                     gate_sb[:, 1:2])
            zt2 = blendp.tile([128, US], BF16, tag="t2")
            nc.any.tensor_scalar_mul(zt2, br_sb[:, 2 * US:3 * US],
                                     gate_sb[:, 2:3])
            zt3 = blendp.tile([128, US], BF16, tag="t3")
            nc.any.tensor_scalar_mul(zt3, br_sb[:, 3 * US:4 * US],
                                     gate_sb[:, 3:4])
            z01 = blendp.tile([128, US], BF16, tag="t0")
            nc.any.tensor_add(z01, zt0, zt1)
            z23 = blendp.tile([128, US], BF16, tag="t2")
            nc.any.tensor_add(z23, zt2, zt3)
            z_sb = blendp.tile([128, US], BF16, tag="t1")
            nc.any.tensor_add(z_sb, z01, z23)
            a_sb = blendp.tile([128, US], BF16, tag="ta")
            nc.any.tensor_scalar_max(a_sb, z_sb, 0.0)
            # blend via degree-4 Horner (coefs pre-scaled by S_BL)
            hp = blendp.tile([128, US], BF16, tag="t2")
            nc.any.tensor_scalar(hp, a_sb, coefs[:, 4:5], coefs[:, 3:4],
                                 ALU.mult, ALU.add)
            hq = blendp.tile([128, US], BF16, tag="t3")
            nc.any.tensor_mul(hq, hp, a_sb)
            hr = blendp.tile([128, US], BF16, tag="t2")
            nc.any.tensor_scalar_add(hr, hq, coefs[:, 2:3])
            hs = blendp.tile([128, US], BF16, tag="t3")
            nc.any.tensor_mul(hs, hr, a_sb)
            ht = blendp.tile([128, US], BF16, tag="t2")
            nc.any.tensor_scalar_add(ht, hs, coefs[:, 1:2])
            hu = blendp.tile([128, US], BF16, tag="t3")
            nc.any.tensor_mul(hu, ht, a_sb)
            blend16 = blendp.tile([128, US], BF16, tag="bb")
            nc.any.tensor_scalar_add(blend16, hu, coefs[:, 0:1])
            csl = slice((i % 4) * 128, (i % 4 + 1) * 128)
            with tc.high_priority():
                for uh in range(2):
                    trb_ps = psum.tile([128, 128], BF16, tag="tr", name="trb")
                    nc.tensor.transpose(trb_ps,
                                        blend16[:, uh * 128:(uh + 1) * 128],
                                        idb)
                    nc.any.tensor_copy(blendT_cur[0][:, uh, csl], trb_ps)

        def issue_ag(ch):
            with tc.high_priority():
                agi = dram_ag.tile([US, 512], FP8, name=f"agi{ch}",
                                   tag=f"agi{ch}")
                nc.scalar.dma_start(
                    out=agi.rearrange("(uh p) c -> p uh c", p=128),
                    in_=blendT_cur[0])
                ago = dram_ag.tile([U, 512], FP8, name=f"ago{ch}",
                                   tag=f"ago{ch}", addr_space="Shared")
                nc.gpsimd.collective_compute(
                    "AllGather", ALU.bypass, replica_groups=[list(range(KC))],
                    ins=[agi.opt()], outs=[ago.opt()])
                bT = p3p.tile([128, UK, 512], FP8, tag="bT", name="bT")
                for uk in range(UK):
                    nc.sync.dma_start(out=bT[:, uk, :],
                                      in_=ago[uk * 128:(uk + 1) * 128, :])
            bTs[ch] = bT
            expTs[ch] = p3p.tile([128, MK, 512], BF16, tag="expT", name="expT")

        def phase3_groups(ch, m0, m1):
            bT, expT = bTs[ch], expTs[ch]
            for mk in range(m0, m1):
                l_ps = psum.tile([128, 512], F32, tag="tr", name="l_ps")
                for j in range(UK // 2):
                    nc.tensor.matmul(l_ps,
                                     rw8[:, 2 * j:2 * j + 2,
                                         mk * 128:(mk + 1) * 128],
                                     bT[:, 2 * j:2 * j + 2, :],
                                     start=(j == 0), stop=(j == UK // 2 - 1),
                                     perf_mode=DR)
                nc.scalar.activation(expT[:, mk, :], l_ps, AF.Exp,
                                     bias=rb_sb[:, mk:mk + 1],
                                     scale=1.0 / (S_BL * S_RW))

        def emit_epilogue(p):
            e_f = p4p.tile([64, MD + 1], F32, tag="ef", name="e_f")
            nc.gpsimd.dma_start(out=e_f, in_=rs_outs[p][:, :])
            s_rec = p4p.tile([64, 1], F32, tag="sr", name="s_rec", bufs=1)
            nc.vector.reciprocal(s_rec, e_f[:, MD:MD + 1])
            y_t = p4p.tile([64, MD], F32, tag="yt", name="y_t", bufs=1)
            nc.any.tensor_scalar_mul(y_t, e_f[:, 0:MD], s_rec)
            nc.gpsimd.dma_start(out=y_d[p * 64:(p + 1) * 64, :], in_=y_t)

        def phase4_sj(ch, sj):
            jsl = slice(sj * 128, (sj + 1) * 128)
            expT = expTs[ch]
            r_ps = psum.tile([128, COLS], F32, tag="br", name="r_ps")
            for (c0, c1) in [(0, 512), (512, 1024), (1024, 1025)]:
                for mk in range(MK):
                    nc.tensor.matmul(r_ps[:, c0:c1], expT[:, mk, jsl],
                                     mem_sb[:, mk, c0:c1],
                                     start=(mk == 0), stop=(mk == MK - 1))
            r_sb = p4p.tile([128, MD + 1], F32, tag="rsb", name="r_sb")
            nc.any.tensor_copy(r_sb, r_ps[:, 0:MD + 1])
            p = ch
            if sj == 0:
                rs_inj = dram_rs.tile([512, MD + 1], F32, name=f"rs_in{p}",
                                      tag=f"rsi{p}")
                phase4_sj.rs_inj = rs_inj
            rs_inj = phase4_sj.rs_inj
            nc.gpsimd.dma_start(out=rs_inj[sj * 128:(sj + 1) * 128, :],
                                in_=r_sb)
            if sj == 3:
                rs_out = dram_rs.tile([64, MD + 1], F32, name=f"rs_out{p}",
                                      tag=f"rso{p}")
                nc.gpsimd.collective_compute(
                    "ReduceScatter", ALU.add, replica_groups=[list(range(KC))],
                    ins=[rs_inj.opt()], outs=[rs_out.opt()])
                rs_outs[p] = rs_out
                if p >= 1:
                    emit_epilogue(n_epi[0])
                    n_epi[0] += 1

        for i in range(NT):
            phase1_tile(i)
            if i % 4 == 3:
                issue_ag(i // 4)
            if i in P3_SCHED:
                phase3_groups(*P3_SCHED[i])
            if i in P4_SCHED:
                phase4_sj(*P4_SCHED[i])

        # tail: phase4(ch2) fills the AG(3) flight window, then phase3/4(ch3)
        for sj in range(4):
            phase4_sj(2, sj)
        phase3_groups(3, 0, 8)
        for sj in range(4):
            phase4_sj(3, sj)
        while n_epi[0] < 4:
            emit_epilogue(n_epi[0])
            n_epi[0] += 1

    nc.compile()
    return nc


def _make_in_maps(inputs):
    FP8NP = ml_dtypes.float8_e4m3
    BF16NP = ml_dtypes.bfloat16
    x = np.asarray(inputs["x"], np.float32)
    w = np.asarray(inputs["w"], np.float32)
    delay = np.asarray(inputs["delay"], np.float32)
    b = np.asarray(inputs["b"], np.float32)
    gate_W = np.asarray(inputs["gate_W"], np.float32)
    gate_b = np.asarray(inputs["gate_b"], np.float32)
    na = np.ascontiguousarray(np.asarray(inputs["neuron_avg"], np.float32))
    cw1 = np.ascontiguousarray(np.asarray(inputs["conn_W1"], np.float32))
    cb1 = np.ascontiguousarray(np.asarray(inputs["conn_b1"], np.float32))
    cw2 = np.asarray(inputs["conn_W2"], np.float32)
    cb2 = np.asarray(inputs["conn_b2"], np.float32)
    mask = np.asarray(inputs["mask"], np.float32)
    actw = np.ascontiguousarray(np.asarray(inputs["act_w"], np.float32))
    read_W = np.asarray(inputs["read_W"], np.float32)
    read_b = np.asarray(inputs["read_b"], np.float32)
    mem = np.asarray(inputs["memory"], np.float32)

    xt8 = np.ascontiguousarray(
        np.clip(x.T, -240, 240)).astype(FP8NP)
    in_maps = []
    for k in range(KC):
        us, ue = k * US, (k + 1) * US
        ms, me = k * MS, (k + 1) * MS
        bias_row = np.concatenate([b[us:ue].T.reshape(-1),
                                   gate_b]).astype(np.float32)
        w16 = np.ascontiguousarray(np.concatenate(
            [w[:, us:ue, :].transpose(0, 2, 1).reshape(D, UBF), gate_W],
            axis=1)).astype(BF16NP)
        d16 = np.ascontiguousarray(
            delay[:, us:ue, :].transpose(0, 2, 1).reshape(D, UBF)
        ).astype(BF16NP)
        rw8 = np.ascontiguousarray(
            np.clip(read_W[:, ms:me] * S_RW, -240, 240)).astype(FP8NP)
        in_maps.append({
            "xt8": xt8,
            "w16": w16,
            "d16": d16,
            "bias": np.ascontiguousarray(bias_row),
            "na": na,
            "cw1": cw1,
            "cb1": cb1,
            "cw2": np.ascontiguousarray(cw2[:, us:ue]),
            "cb2": np.ascontiguousarray(cb2[us:ue]),
            "maskv": np.ascontiguousarray(mask[us:ue]),
            "actw": actw,
            "rw8": rw8,
            "rb": np.ascontiguousarray(read_b[ms:me]),
            "mem16": np.ascontiguousarray(mem[ms:me, :]).astype(BF16NP),
            "cmat": _CMAT,
        })
    return in_maps


def kernel(**inputs) -> np.ndarray:
    if "nc" not in _cache:
        _cache["nc"] = _build()
    nc = _cache["nc"]
    in_maps = _make_in_maps(inputs)
    res = run_bass_kernel_spmd(nc, in_maps, core_ids=list(range(KC)))
    out = np.empty((N, MD), np.float32)
    for k in range(KC):
        yk = res.results[k]["y"]
        for p in range(4):
            out[p * 512 + k * 64:p * 512 + (k + 1) * 64] = \
                yk[p * 64:(p + 1) * 64]
    return out
